# revision 14
# baseline (speedup 1.0000x reference)
"""Trainium2 Bass kernel for nn_AttenCross (sparse_attention).

reference:
    scores = einsum('bqd,bkd->bqk', Q, D) / sqrt(H)
    scores = where(doc_mask==0, -9999, scores)
    attn   = softmax(scores, -1)
    out    = sum over k of (attn * sim), then sum over q -> (B, 1)

Strategy (8 cores, data-parallel over batch, 2 batches/core).
Host-side prep (sharding/layout/encoding only, exact for any inputs):
slice per core; transpose Q and D to [H, L] layout (the PE contracts
over partitions); apply the doc mask by zeroing masked doc rows of D
and masked sim columns; pass the per-batch masked count.  With D rows
zeroed, masked scores are exactly 0 so exp gives exactly 1: subtracting
the masked count from the exp row-sum reproduces the exact softmax
denominator, and masked sim columns are zero so they add nothing to the
numerator.  (No row-max subtraction: scores ~ N(0,1); softmax is
shift-invariant.)

Device, per batch:
  - round Q^T/D^T to fp32r (PE full-rate fp32 mode; inputs are rounded
    to 12-bit mantissa, exact multiply, fp32 accumulate).
  - per q-tile (128 queries): fp32r QK^T matmuls into PSUM; ACT computes
    E = exp(scale*psum) into SBUF with fused accum_out row-sums (den).
  - DVE: fused multiply P = E * sim (fp32r out), den fixup, reciprocal.
  - PE: column-sum matmuls with 1/den as the stationary operand (only
    column 0 of a [128,128] fp32r tile nonzero) accumulate
    sum_q P[q,k]/den_q into one [128,512] PSUM bank across all q-tiles
    and segments; epilogue reduces that bank to the scalar output.
Output per core: [2, 1]; host stacks to [16, 1] fp32.
"""

import numpy as np

import concourse.bacc as bacc
import concourse.tile as tile
import concourse.mybir as mybir
from concourse.bass_utils import run_bass_kernel_spmd

B, QL, DL, H = 16, 1024, 4096, 128
NCORES = 8
BPC = B // NCORES  # batches per core
QT_N = QL // 128  # 8 q-tiles per batch
SEG = 512
NSEG = DL // SEG  # 8
CH = 1024
NCH = DL // CH  # 4
SCALE = 1.0 / float(np.sqrt(H))

f32 = mybir.dt.float32
f32r = mybir.dt.float32r

_CACHED = {}


def _build():
    nc = bacc.Bacc("TRN2", target_bir_lowering=False, debug=False)

    qtd = nc.dram_tensor("qt", [BPC, H, QL], f32, kind="ExternalInput")
    dtd = nc.dram_tensor("dt", [BPC, H, DL], f32, kind="ExternalInput")
    sd = nc.dram_tensor("s", [BPC, QL, DL], f32, kind="ExternalInput")
    cntd = nc.dram_tensor("cnt", [BPC, 1], f32, kind="ExternalInput")
    outd = nc.dram_tensor("o", [BPC, 1], f32, kind="ExternalOutput")

    with tile.TileContext(nc) as tc:
        with (
            tc.tile_pool(name="const", bufs=1) as const,
            tc.tile_pool(name="raw", bufs=1) as raw,
            tc.tile_pool(name="b2", bufs=2) as b2,
            tc.tile_pool(name="dtp", bufs=2) as dtp,
            tc.tile_pool(name="simp", bufs=3) as simp,
            tc.tile_pool(name="ep", bufs=3) as ep,
            tc.tile_pool(name="pp", bufs=2) as pp,
            tc.tile_pool(name="small", bufs=4) as small,
            tc.tile_pool(name="bsm", bufs=2) as bsm,
            tc.tile_pool(name="pscore", bufs=3, space="PSUM") as pscore,
            tc.tile_pool(name="pacc", bufs=1, space="PSUM") as pacc,
            tc.tile_pool(name="ptp", bufs=1, space="PSUM") as ptp,
        ):
            ones128 = const.tile([128, 1], f32, tag="ones128")
            nc.vector.memset(ones128, 1.0)
            z128 = const.tile([128, 128], f32, tag="z128")
            nc.vector.memset(z128, 0.0)
            r128a = const.tile([128, 128], f32r, tag="r128a")
            nc.vector.tensor_copy(r128a, z128)
            r128b = const.tile([128, 128], f32r, tag="r128b")
            nc.vector.tensor_copy(r128b, z128)

            for b in range(BPC):
                # ---- per-batch loads + fp32r rounding (dt split for ramp) ----
                qtraw = raw.tile([128, QL], f32, tag="qtraw")
                nc.sync.dma_start(qtraw, qtd.ap()[b])
                qt = b2.tile([128, QL], f32r, tag="qt")
                nc.vector.tensor_copy(qt, qtraw)
                dtraw = raw.tile([128, DL], f32, tag="dtraw")
                dt = dtp.tile([128, DL], f32r, tag="dt")
                half = DL // 2
                for hh in range(2):
                    sl = slice(hh * half, (hh + 1) * half)
                    nc.sync.dma_start(dtraw[:, sl], dtd.ap()[b][:, sl])
                    nc.vector.tensor_copy(dt[:, sl], dtraw[:, sl])

                # den correction: crep[q] = masked count, replicated via
                # partition-broadcast DMA
                crep = bsm.tile([128, 1], f32, tag="crep")
                cnt_ap = cntd.ap()[b : b + 1, :]
                import concourse.bass as _bass
                cnt_bcast = _bass.AP(
                    tensor=cnt_ap.tensor,
                    offset=cnt_ap.offset,
                    ap=[[0, 128], [1, 1]],
                )
                nc.sync.dma_start(crep, cnt_bcast)

                # column-sum accumulator: row 0 collects sum_q P[q,k]/den_q
                acc = pacc.tile([128, SEG], f32, tag="acc")

                # ---- q-tiles ----
                for t in range(QT_N):
                    sim_t = simp.tile([128, DL], f32, tag="sim")
                    nc.sync.dma_start(
                        sim_t[:, :half],
                        sd.ap()[b, t * 128 : (t + 1) * 128, :half],
                    )
                    nc.sync.dma_start(
                        sim_t[:, half:],
                        sd.ap()[b, t * 128 : (t + 1) * 128, half:],
                    )
                    e_t = ep.tile([128, DL], f32, tag="e")
                    den4 = small.tile([128, NCH], f32, tag="den4")
                    for c in range(NCH):
                        psc = pscore.tile([128, CH], f32, tag="sc")
                        for hh in range(CH // SEG):
                            off = c * CH + hh * SEG
                            nc.tensor.matmul(
                                psc[:, hh * SEG : (hh + 1) * SEG],
                                qt[:, t * 128 : (t + 1) * 128],
                                dt[:, off : off + SEG],
                                start=True,
                                stop=True,
                            )
                        nc.scalar.activation(
                            out=e_t[:, c * CH : (c + 1) * CH],
                            in_=psc,
                            func=mybir.ActivationFunctionType.Exp,
                            scale=SCALE,
                            accum_out=den4[:, c : c + 1],
                        )

                    den = small.tile([128, 1], f32, tag="den")
                    nc.vector.reduce_sum(den, den4, axis=mybir.AxisListType.X)
                    dent = small.tile([128, 1], f32, tag="dent")
                    nc.vector.tensor_scalar(
                        dent, den, crep, None, mybir.AluOpType.subtract
                    )
                    rv = small.tile([128, 1], f32, tag="rv")
                    nc.vector.reciprocal(rv, dent)
                    r128 = r128a if t % 2 == 0 else r128b
                    nc.vector.tensor_copy(r128[:, 0:1], rv)

                    p_t = pp.tile([128, DL], f32r, tag="p")
                    for qq in range(2):
                        lo, hi = qq * half, (qq + 1) * half
                        nc.vector.tensor_tensor(
                            p_t[:, lo:hi], e_t[:, lo:hi], sim_t[:, lo:hi],
                            mybir.AluOpType.mult,
                        )

                    for j in range(NSEG):
                        nc.tensor.matmul(
                            acc,
                            r128,
                            p_t[:, j * SEG : (j + 1) * SEG],
                            start=(t == 0 and j == 0),
                            stop=(t == QT_N - 1 and j == NSEG - 1),
                            skip_group_check=True,
                        )

                # ---- batch epilogue ----
                red128 = bsm.tile([128, 1], f32, tag="red128")
                nc.vector.reduce_sum(red128, acc, axis=mybir.AxisListType.X)
                ps_o = ptp.tile([1, 1], f32, tag="tp")
                nc.tensor.matmul(ps_o, red128, ones128, start=True, stop=True)
                out_sb = bsm.tile([1, 1], f32, tag="out_sb")
                nc.vector.tensor_copy(out_sb, ps_o)
                nc.sync.dma_start(outd.ap()[b : b + 1, :], out_sb)

    nc.compile()
    return nc


def kernel(**inputs: np.ndarray) -> np.ndarray:
    if "nc" not in _CACHED:
        _CACHED["nc"] = _build()
    nc = _CACHED["nc"]

    q = np.asarray(inputs["query_input"], dtype=np.float32)
    d = np.asarray(inputs["doc_input"], dtype=np.float32)
    s = np.asarray(inputs["sim_matrix"], dtype=np.float32)
    dm = (np.asarray(inputs["doc_mask"]) != 0).astype(np.float32)  # [B, DL]

    qt = np.ascontiguousarray(np.swapaxes(q, 1, 2))  # [B, H, QL]
    dt = np.ascontiguousarray(np.swapaxes(d * dm[:, :, None], 1, 2))  # [B, H, DL]
    sm = np.ascontiguousarray(s * dm[:, None, :])  # [B, QL, DL]
    cnt = (DL - dm.sum(axis=1, keepdims=True)).astype(np.float32)  # [B, 1]

    in_maps = []
    for c in range(NCORES):
        lo, hi = c * BPC, (c + 1) * BPC
        in_maps.append(
            {
                "qt": qt[lo:hi],
                "dt": dt[lo:hi],
                "s": sm[lo:hi],
                "cnt": cnt[lo:hi],
            }
        )

    res = None
    for attempt in range(3):
        try:
            res = run_bass_kernel_spmd(nc, in_maps, core_ids=list(range(NCORES)))
            break
        except Exception:
            if attempt == 2:
                raise
    out = np.concatenate([res.results[c]["o"] for c in range(NCORES)], axis=0)
    return out.astype(np.float32)


# revision 15
# speedup vs baseline: 1.0797x; 1.0797x over previous
"""Trainium2 Bass kernel for nn_AttenCross (sparse_attention).

reference:
    scores = einsum('bqd,bkd->bqk', Q, D) / sqrt(H)
    scores = where(doc_mask==0, -9999, scores)
    attn   = softmax(scores, -1)
    out    = sum over k of (attn * sim), then sum over q -> (B, 1)

Strategy (8 cores, data-parallel over batch, 2 batches/core).
Host-side prep (sharding/layout/encoding only, exact for any inputs):
slice per core; transpose Q and D to [H, L] layout (the PE contracts
over partitions); apply the doc mask by zeroing masked doc rows of D
and masked sim columns; pass the per-batch masked count.  With D rows
zeroed, masked scores are exactly 0 so exp gives exactly 1: subtracting
the masked count from the exp row-sum reproduces the exact softmax
denominator, and masked sim columns are zero so they add nothing to the
numerator.  (No row-max subtraction: scores ~ N(0,1); softmax is
shift-invariant.)

Device, per batch:
  - round Q^T/D^T to fp32r (PE full-rate fp32 mode; inputs are rounded
    to 12-bit mantissa, exact multiply, fp32 accumulate).
  - per q-tile (128 queries): fp32r QK^T matmuls into PSUM; ACT computes
    E = exp(scale*psum) into SBUF with fused accum_out row-sums (den).
  - DVE: fused multiply P = E * sim (fp32r out), den fixup, reciprocal.
  - PE: column-sum matmuls with 1/den as the stationary operand (only
    column 0 of a [128,128] fp32r tile nonzero) accumulate
    sum_q P[q,k]/den_q into one [128,512] PSUM bank across all q-tiles
    and segments; epilogue reduces that bank to the scalar output.
Output per core: [2, 1]; host stacks to [16, 1] fp32.
"""

import numpy as np

import concourse.bacc as bacc
import concourse.tile as tile
import concourse.mybir as mybir
from concourse.bass_utils import run_bass_kernel_spmd

B, QL, DL, H = 16, 1024, 4096, 128
NCORES = 8
BPC = B // NCORES  # batches per core
QT_N = QL // 128  # 8 q-tiles per batch
SEG = 512
NSEG = DL // SEG  # 8
CH = 1024
NCH = DL // CH  # 4
SCALE = 1.0 / float(np.sqrt(H))

f32 = mybir.dt.float32
f32r = mybir.dt.float32r

_CACHED = {}


def _build():
    nc = bacc.Bacc("TRN2", target_bir_lowering=False, debug=False)

    qtd = nc.dram_tensor("qt", [BPC, H, QL], f32, kind="ExternalInput")
    dtd = nc.dram_tensor("dt", [BPC, H, DL], f32, kind="ExternalInput")
    sd = nc.dram_tensor("s", [BPC, QL, DL], f32, kind="ExternalInput")
    cntd = nc.dram_tensor("cnt", [BPC, 1], f32, kind="ExternalInput")
    outd = nc.dram_tensor("o", [BPC, 1], f32, kind="ExternalOutput")

    with tile.TileContext(nc) as tc:
        with (
            tc.tile_pool(name="const", bufs=1) as const,
            tc.tile_pool(name="raw", bufs=1) as raw,
            tc.tile_pool(name="b2", bufs=2) as b2,
            tc.tile_pool(name="dtp", bufs=2) as dtp,
            tc.tile_pool(name="simp", bufs=3) as simp,
            tc.tile_pool(name="ep", bufs=2) as ep,
            tc.tile_pool(name="pp", bufs=2) as pp,
            tc.tile_pool(name="small", bufs=4) as small,
            tc.tile_pool(name="bsm", bufs=2) as bsm,
            tc.tile_pool(name="pscore", bufs=3, space="PSUM") as pscore,
            tc.tile_pool(name="pacc", bufs=1, space="PSUM") as pacc,
            tc.tile_pool(name="ptp", bufs=1, space="PSUM") as ptp,
        ):
            ones128 = const.tile([128, 1], f32, tag="ones128")
            nc.vector.memset(ones128, 1.0)
            z128 = const.tile([128, 128], f32, tag="z128")
            nc.vector.memset(z128, 0.0)
            r128a = const.tile([128, 128], f32r, tag="r128a")
            nc.vector.tensor_copy(r128a, z128)
            r128b = const.tile([128, 128], f32r, tag="r128b")
            nc.vector.tensor_copy(r128b, z128)

            for b in range(BPC):
                # ---- per-batch loads + fp32r rounding (dt split for ramp) ----
                qtraw = raw.tile([128, QL], f32, tag="qtraw")
                nc.sync.dma_start(qtraw, qtd.ap()[b])
                qt = b2.tile([128, QL], f32r, tag="qt")
                nc.vector.tensor_copy(qt, qtraw)
                dtraw = raw.tile([128, DL], f32, tag="dtraw")
                dt = dtp.tile([128, DL], f32r, tag="dt")
                half = DL // 2
                for hh in range(2):
                    sl = slice(hh * half, (hh + 1) * half)
                    nc.sync.dma_start(dtraw[:, sl], dtd.ap()[b][:, sl])
                    nc.vector.tensor_copy(dt[:, sl], dtraw[:, sl])

                # den correction: crep[q] = masked count, replicated via
                # partition-broadcast DMA
                crep = bsm.tile([128, 1], f32, tag="crep")
                cnt_ap = cntd.ap()[b : b + 1, :]
                import concourse.bass as _bass
                cnt_bcast = _bass.AP(
                    tensor=cnt_ap.tensor,
                    offset=cnt_ap.offset,
                    ap=[[0, 128], [1, 1]],
                )
                nc.sync.dma_start(crep, cnt_bcast)

                # column-sum accumulator: row 0 collects sum_q P[q,k]/den_q
                acc = pacc.tile([128, SEG], f32, tag="acc")

                # ---- q-tiles ----
                for t in range(QT_N):
                    sim_t = simp.tile([128, DL], f32, tag="sim")
                    nc.sync.dma_start(
                        sim_t[:, :half],
                        sd.ap()[b, t * 128 : (t + 1) * 128, :half],
                    )
                    nc.sync.dma_start(
                        sim_t[:, half:],
                        sd.ap()[b, t * 128 : (t + 1) * 128, half:],
                    )
                    e_t = ep.tile([128, DL], f32, tag="e")
                    den4 = small.tile([128, NCH], f32, tag="den4")
                    for c in range(NCH):
                        psc = pscore.tile([128, CH], f32, tag="sc")
                        for hh in range(CH // SEG):
                            off = c * CH + hh * SEG
                            nc.tensor.matmul(
                                psc[:, hh * SEG : (hh + 1) * SEG],
                                qt[:, t * 128 : (t + 1) * 128],
                                dt[:, off : off + SEG],
                                start=True,
                                stop=True,
                            )
                        nc.scalar.activation(
                            out=e_t[:, c * CH : (c + 1) * CH],
                            in_=psc,
                            func=mybir.ActivationFunctionType.Exp,
                            scale=SCALE,
                            accum_out=den4[:, c : c + 1],
                        )

                    den = small.tile([128, 1], f32, tag="den")
                    nc.vector.reduce_sum(den, den4, axis=mybir.AxisListType.X)
                    dent = small.tile([128, 1], f32, tag="dent")
                    nc.vector.tensor_scalar(
                        dent, den, crep, None, mybir.AluOpType.subtract
                    )
                    rv = small.tile([128, 1], f32, tag="rv")
                    nc.vector.reciprocal(rv, dent)
                    r128 = r128a if t % 2 == 0 else r128b
                    nc.vector.tensor_copy(r128[:, 0:1], rv)

                    p_t = pp.tile([128, DL], f32r, tag="p")
                    for qq in range(4):
                        lo, hi = qq * (DL // 4), (qq + 1) * (DL // 4)
                        nc.vector.tensor_tensor(
                            p_t[:, lo:hi], e_t[:, lo:hi], sim_t[:, lo:hi],
                            mybir.AluOpType.mult,
                        )

                    for j in range(NSEG):
                        nc.tensor.matmul(
                            acc,
                            r128,
                            p_t[:, j * SEG : (j + 1) * SEG],
                            start=(t == 0 and j == 0),
                            stop=(t == QT_N - 1 and j == NSEG - 1),
                            skip_group_check=True,
                        )

                # ---- batch epilogue ----
                red128 = bsm.tile([128, 1], f32, tag="red128")
                nc.vector.reduce_sum(red128, acc, axis=mybir.AxisListType.X)
                ps_o = ptp.tile([1, 1], f32, tag="tp")
                nc.tensor.matmul(ps_o, red128, ones128, start=True, stop=True)
                out_sb = bsm.tile([1, 1], f32, tag="out_sb")
                nc.vector.tensor_copy(out_sb, ps_o)
                nc.sync.dma_start(outd.ap()[b : b + 1, :], out_sb)

    nc.compile()
    return nc


def kernel(**inputs: np.ndarray) -> np.ndarray:
    if "nc" not in _CACHED:
        _CACHED["nc"] = _build()
    nc = _CACHED["nc"]

    q = np.asarray(inputs["query_input"], dtype=np.float32)
    d = np.asarray(inputs["doc_input"], dtype=np.float32)
    s = np.asarray(inputs["sim_matrix"], dtype=np.float32)
    dm = (np.asarray(inputs["doc_mask"]) != 0).astype(np.float32)  # [B, DL]

    qt = np.ascontiguousarray(np.swapaxes(q, 1, 2))  # [B, H, QL]
    dt = np.ascontiguousarray(np.swapaxes(d * dm[:, :, None], 1, 2))  # [B, H, DL]
    sm = np.ascontiguousarray(s * dm[:, None, :])  # [B, QL, DL]
    cnt = (DL - dm.sum(axis=1, keepdims=True)).astype(np.float32)  # [B, 1]

    in_maps = []
    for c in range(NCORES):
        lo, hi = c * BPC, (c + 1) * BPC
        in_maps.append(
            {
                "qt": qt[lo:hi],
                "dt": dt[lo:hi],
                "s": sm[lo:hi],
                "cnt": cnt[lo:hi],
            }
        )

    res = None
    for attempt in range(3):
        try:
            res = run_bass_kernel_spmd(nc, in_maps, core_ids=list(range(NCORES)))
            break
        except Exception:
            if attempt == 2:
                raise
    out = np.concatenate([res.results[c]["o"] for c in range(NCORES)], axis=0)
    return out.astype(np.float32)
